# revision 26
# baseline (speedup 1.0000x reference)
"""Bidirectional Mamba block on 8 Trainium2 NeuronCores (Bass/Tile).

Data-parallel over batch: B=16 -> 2 per core; weights replicated; host gathers.
Per-core layout is feature-major ([feature_partitions, tokens]) with tokens =
batch-major concatenation of the 2 local sequences (t = b*512 + l).

Engines:
  PE   - all projections (weights stationary as lhsT), depthwise causal conv as
         4 accumulating diag-matmuls over shifted views, y = sum_n h_n*C_n
         accumulated in PSUM via identity matmuls (all 4 d-tiles).
  ACT  - silu via the native Silu table; softplus = ln(exp(.)+1);
         rsqrt = exp(-0.5*ln(.)); dA_n = exp(delta * A[:,n]) with per-partition
         scale; PSUM->SBUF copies.
  DVE  - selective scan via tensor_tensor_scan; bx/p elementwise muls (bf16 2x);
         the backward layer feeds the scan with reversed access patterns.
  DMA  - per-token B/C rows are bounced through a DRAM scratch tile and
         partition-broadcast to 128 partitions by pure DMA reads (no engine
         time), replacing one-hot selector matmuls + ACT copies.
"""

import numpy as np

# ---- problem constants (hardcoded per contract) ----
B, L, DM = 16, 512, 256
DI, N, R, KC = 512, 16, 16, 4
NCORES = 8
BL = B // NCORES          # local batch
TOK = BL * L              # 1024 tokens per core
DT_TILES = DI // 128      # 4
MT = DM // 128            # 2
F32_np = np.float32

# ---- dtype knobs for the scan path ----
import ml_dtypes
BF16_np = ml_dtypes.bfloat16

CFG = dict(
    DA="bf16",     # dA (scan decay operand)
    DELTA="bf16",  # delta resident
    W="bf16",      # w = delta*xs (scan drive factor)
    H="bf16",      # scan output h
    REP="bf16",    # B_rep / C_rep broadcast tiles
    P="bf16",      # products h*C
    SZ="bf16",     # silu(z) gate
    XS="bf16",     # conv-silu output / gate buffer
    PROBE="",      # timing probes: shrink a stage's work (breaks numerics)
    P_POOL_DT=(),       # d-tiles whose p=h*C mul runs on GPSIMD
    BX_POOL_DT=(),      # d-tiles whose bx=w*B mul runs on GPSIMD
    GP_GATE=0,          # gate mul xs*sz on GPSIMD
)

_BUILD_CACHE = {}


# ======================================================================
# host-side weight preparation
# ======================================================================

def _prep_layer_weights(inw, convw, convb, xprojw, dtw, dtb, Alog, Dp, outw, normw):
    """Fold/reshape one mamba layer's weights into device layouts."""
    out = {}
    # in_proj with rmsnorm weight folded into rows: [128, 2, 1024]
    w = (np.asarray(normw)[:, None] * np.asarray(inw)).astype(F32_np)
    out["inw"] = np.ascontiguousarray(w.reshape(2, 128, 2 * DI).transpose(1, 0, 2)).astype(BF16_np)
    # conv diag matrices: [128, 16(dt*4+k), 128]
    cd = np.zeros((128, DT_TILES * KC, 128), F32_np)
    cw = np.asarray(convw).astype(F32_np)  # (KC, 1, DI)
    for dt in range(DT_TILES):
        for k in range(KC):
            idx = np.arange(128)
            cd[idx, dt * KC + k, idx] = cw[k, 0, dt * 128 + idx]
    out["convd"] = np.ascontiguousarray(cd).astype(BF16_np)
    out["convbn"] = np.ascontiguousarray(
        (-np.asarray(convb).astype(F32_np)).reshape(DT_TILES, 128, 1).transpose(1, 0, 2))
    out["convb"] = np.ascontiguousarray(
        np.asarray(convb).astype(F32_np).reshape(DT_TILES, 128, 1).transpose(1, 0, 2))
    # xproj padded so delta_raw/B/C land at partitions 0/32/64: [128, 4, 96]
    xp = np.zeros((DI, 96), F32_np)
    xpw = np.asarray(xprojw).astype(F32_np)
    xp[:, 0:R] = xpw[:, 0:R]
    xp[:, 32:32 + N] = xpw[:, R:R + N]
    xp[:, 64:64 + N] = xpw[:, R + N:R + 2 * N]
    out["xpw"] = np.ascontiguousarray(xp.reshape(DT_TILES, 128, 96).transpose(1, 0, 2)).astype(BF16_np)
    out["dtw"] = np.ascontiguousarray(np.asarray(dtw).astype(F32_np)).astype(BF16_np)          # (16, 512)
    out["dtb"] = np.ascontiguousarray(
        np.asarray(dtb).astype(F32_np).reshape(DT_TILES, 128, 1).transpose(1, 0, 2))
    A = (-np.exp(np.asarray(Alog).astype(np.float64))).astype(F32_np)          # (512, 16)
    out["A"] = np.ascontiguousarray(A.reshape(DT_TILES, 128, N).transpose(1, 0, 2))
    out["Dp"] = np.ascontiguousarray(
        np.asarray(Dp).astype(F32_np).reshape(DT_TILES, 128, 1).transpose(1, 0, 2))
    out["outw"] = np.ascontiguousarray(
        np.asarray(outw).astype(F32_np).reshape(DT_TILES, 128, DM).transpose(1, 0, 2)).astype(BF16_np)
    return out


def _prep_shared_weights(proj_w, proj_b, ln_g, ln_b):
    out = {}
    out["projw"] = np.ascontiguousarray(
        np.asarray(proj_w).astype(F32_np).reshape(4, 128, DM).transpose(1, 0, 2)).astype(BF16_np)
    out["projb"] = np.ascontiguousarray(
        np.asarray(proj_b).astype(F32_np).reshape(MT, 128, 1).transpose(1, 0, 2))
    out["lng"] = np.ascontiguousarray(
        np.asarray(ln_g).astype(F32_np).reshape(MT, 128, 1).transpose(1, 0, 2))
    out["lnb"] = np.ascontiguousarray(
        np.asarray(ln_b).astype(F32_np).reshape(MT, 128, 1).transpose(1, 0, 2))
    return out


# ======================================================================
# device program
# ======================================================================

def _build(loop_k=1, cfg=None, variant="full"):
    cfg = dict(CFG if cfg is None else cfg)
    key = (loop_k, variant, tuple(sorted(cfg.items())))
    if key in _BUILD_CACHE:
        return _BUILD_CACHE[key]

    import concourse.bacc as bacc
    import concourse.mybir as mybir
    import concourse.tile as tile

    F32 = mybir.dt.float32
    BF16 = mybir.dt.bfloat16
    AF = mybir.ActivationFunctionType
    ALU = mybir.AluOpType
    AX = mybir.AxisListType

    def dt_of(kname):
        return F32 if cfg[kname] == "f32" else BF16

    nc = bacc.Bacc("TRN2", target_bir_lowering=False, debug=False)

    def din(name, shape, dt=None):
        return nc.dram_tensor(name, list(shape), dt or F32, kind="ExternalInput").ap()

    # --- DRAM I/O ---
    xT_d = din("xT", (DM, TOK))
    lw_d = {}
    for s in ("f", "b"):
        lw_d[s] = {
            "inw": din(f"{s}_inw", (128, 2, 2 * DI), BF16),
            "convd": din(f"{s}_convd", (128, DT_TILES * KC, 128), BF16),
            "convbn": din(f"{s}_convbn", (128, DT_TILES, 1)),
            "convb": din(f"{s}_convb", (128, DT_TILES, 1)),
            "xpw": din(f"{s}_xpw", (128, DT_TILES, 96), BF16),
            "dtw": din(f"{s}_dtw", (16, DI), BF16),
            "dtb": din(f"{s}_dtb", (128, DT_TILES, 1)),
            "A": din(f"{s}_A", (128, DT_TILES, N)),
            "Dp": din(f"{s}_Dp", (128, DT_TILES, 1)),
            "outw": din(f"{s}_outw", (128, DT_TILES, DM), BF16),
        }
    projw_d = din("projw", (128, 4, DM), BF16)
    projb_d = din("projb", (128, MT, 1))
    lng_d = din("lng", (128, MT, 1))
    lnb_d = din("lnb", (128, MT, 1))
    outT_d = nc.dram_tensor("outT", [DM, TOK], F32, kind="ExternalOutput").ap()

    PAD = KC - 1  # 3
    CONVW = 2 * PAD + L  # padded per-batch row length 518

    with tile.TileContext(nc) as tc:
        from contextlib import ExitStack
        with ExitStack() as ctx:
            wpool = ctx.enter_context(tc.tile_pool(name="wpool", bufs=1))
            pers = ctx.enter_context(tc.tile_pool(name="pers", bufs=1))
            work = ctx.enter_context(tc.tile_pool(name="work", bufs=1))
            rep = ctx.enter_context(tc.tile_pool(name="rep", bufs=3))
            scanw = ctx.enter_context(tc.tile_pool(name="scanw", bufs=3))
            dpool = ctx.enter_context(tc.tile_pool(name="dpool", bufs=1, space="DRAM"))

            def body():
                # ---- load shared weights ----
                projw_t = wpool.tile([128, 4, DM], BF16, tag="projw", name="projw")
                nc.sync.dma_start(projw_t[:], projw_d[:])
                projb_t = wpool.tile([128, MT, 1], F32, tag="projb", name="projb")
                nc.sync.dma_start(projb_t[:], projb_d[:])
                lng_t = wpool.tile([128, MT, 1], F32, tag="lng", name="lng")
                nc.sync.dma_start(lng_t[:], lng_d[:])
                lnb_t = wpool.tile([128, MT, 1], F32, tag="lnb", name="lnb")
                nc.sync.dma_start(lnb_t[:], lnb_d[:])

                xT = []
                for m in range(MT):
                    t = pers.tile([128, TOK], F32, tag=f"xT{m}", name=f"xT{m}")
                    nc.sync.dma_start(t[:], xT_d[m * 128:(m + 1) * 128, :])
                    xT.append(t)

                # ---- shared RMSNorm: xn = x * rsqrt(mean(x^2) + eps) ----
                xn = []
                with tc.tile_pool(name="prms", bufs=1, space="PSUM") as prms:
                    ones_col = wpool.tile([128, 1], F32, tag="ones_col", name="ones_col")
                    nc.vector.memset(ones_col[:], 1.0)
                    ss_ps = prms.tile([1, TOK], F32, tag="ss", name="ss")
                    for fh in range(2):
                        fs = slice(fh * 512, (fh + 1) * 512)
                        for m in range(MT):
                            sq = work.tile([128, 512], F32, tag="sqtmp", name="rms_sq")
                            nc.scalar.square(sq[:], xT[m][:, fs])
                            nc.tensor.matmul(ss_ps[:, fs],ones_col[:],sq[:],
                                             start=(m == 0), stop=(m == MT - 1))
                    # rs = exp(-0.5 * ln(ss/DM + eps))
                    eps1 = wpool.tile([1, 1], F32, tag="eps1", name="eps1")
                    nc.vector.memset(eps1[:], 1e-5)
                    rs_row = work.tile([1, TOK], F32, tag="rowtmp", name="rs_row")
                    nc.scalar.activation(rs_row[:], ss_ps[:], AF.Ln,
                                         scale=1.0 / DM, bias=eps1[:, 0:1])
                    nc.scalar.activation(rs_row[:], rs_row[:], AF.Exp, scale=-0.5)
                    # broadcast rs to 128 partitions via DRAM-bounce DMA
                    rs_d = dpool.tile([1, TOK], F32, tag="rs_d", name="rs_d")
                    nc.sync.dma_start(rs_d[:], rs_row[:])
                    rs_bc = pers.tile([128, TOK], F32, tag="rs_bc", name="rs_bc")
                    nc.sync.dma_start(rs_bc[:], rs_d[0:1, :].partition_broadcast(128))
                    for m in range(MT):
                        t = pers.tile([128, TOK], BF16, tag=f"xn{m}", name=f"xn{m}")
                        nc.vector.tensor_mul(t[:], xT[m][:], rs_bc[:])
                        xn.append(t)

                # ---- one mamba layer ----
                def mamba_layer(s, reverse):
                    W = lw_d[s]
                    inw_t = wpool.tile([128, 2, 2 * DI], BF16, tag="inw", name="inw")
                    nc.sync.dma_start(inw_t[:], W["inw"][:])
                    convd_t = wpool.tile([128, DT_TILES * KC, 128], BF16, tag="convd", name="convd")
                    nc.sync.dma_start(convd_t[:], W["convd"][:])
                    convb_t = wpool.tile([128, DT_TILES, 1], F32, tag="convb", name="convb")
                    nc.sync.dma_start(convb_t[:], W["convb"][:])
                    xpw_t = wpool.tile([128, DT_TILES, 96], BF16, tag="xpw", name="xpw")
                    nc.sync.dma_start(xpw_t[:], W["xpw"][:])
                    dtw_t = wpool.tile([16, DI], BF16, tag="dtw", name="dtw")
                    nc.sync.dma_start(dtw_t[:], W["dtw"][:])
                    dtb_t = wpool.tile([128, DT_TILES, 1], F32, tag="dtb", name="dtb")
                    nc.sync.dma_start(dtb_t[:], W["dtb"][:])
                    A_t = wpool.tile([128, DT_TILES, N], F32, tag="A", name="A")
                    nc.sync.dma_start(A_t[:], W["A"][:])
                    Dp_t = wpool.tile([128, DT_TILES, 1], F32, tag="Dp", name="Dp")
                    nc.sync.dma_start(Dp_t[:], W["Dp"][:])
                    outw_t = wpool.tile([128, DT_TILES, DM], BF16, tag="outw", name="outw")
                    nc.sync.dma_start(outw_t[:], W["outw"][:])

                    xmpad = []
                    sz = []
                    xs = []
                    for dt in range(DT_TILES):
                        t = pers.tile([128, BL, CONVW], BF16, tag=f"xmpad{dt}", name=f"xmpad{dt}")
                        nc.vector.memset(t[:, :, 0:PAD], 0.0)
                        nc.vector.memset(t[:, :, PAD + L:CONVW], 0.0)
                        xmpad.append(t)
                        sz.append(pers.tile([128, TOK], dt_of("SZ"), tag=f"sz{dt}", name=f"sz{dt}"))
                        xs.append(pers.tile([128, TOK], dt_of("XS"), tag=f"xs{dt}", name=f"xs{dt}"))

                    # ---- in_proj ----
                    with tc.tile_pool(name="pp", bufs=4, space="PSUM") as pp:
                        for m in range(8):
                            for fh in range(2):
                                fs = slice(fh * 512, (fh + 1) * 512)
                                ps = pp.tile([128, 512], F32, tag="pp", name="pp")
                                for ks in range(2):
                                    nc.tensor.matmul(
                                        ps[:],inw_t[:, ks, m * 128:(m + 1) * 128],xn[ks][:, fs], start=(ks == 0), stop=(ks == 1))
                                if m < 4:
                                    # xm -> padded conv buffer (fh == local batch idx)
                                    nc.scalar.copy(xmpad[m][:, fh, PAD:PAD + L], ps[:])
                                else:
                                    zdt = m - 4
                                    nc.scalar.activation(sz[zdt][:, fs], ps[:], AF.Silu)

                        # ---- depthwise causal conv + silu ----
                        for dt in range(DT_TILES):
                            for b in range(BL):
                                ps = pp.tile([128, 512], F32, tag="pp", name="pp")
                                for k in range(KC):
                                    off = k if not reverse else (2 * PAD - k)
                                    nc.tensor.matmul(
                                        ps[:],convd_t[:, dt * KC + k, :],xmpad[dt][:, b, off:off + L],
                                        start=(k == 0), stop=(k == KC - 1))
                                bs = slice(b * L, (b + 1) * L)
                                nc.scalar.activation(xs[dt][:, bs], ps[:], AF.Silu,
                                                     bias=convb_t[:, dt, 0:1])

                    if cfg["PROBE"] == "stop_conv":
                        return [xs[0], xs[1]]
                    # ---- xproj -> delta_raw / Brows / Crows ----
                    dbc = pers.tile([16, 2, TOK], BF16, tag="dbc", name="dbc")
                    draw_t = work.tile([16, TOK], BF16, tag="draw", name="draw_t")
                    draw = draw_t[:, :]
                    Brows = dbc[:, 0, :]
                    Crows = dbc[:, 1, :]
                    with tc.tile_pool(name="pxp", bufs=1, space="PSUM") as pxp:
                        psx = pxp.tile([96, TOK], F32, tag="pxp", name="pxp")
                        for fh in range(2):
                            fs = slice(fh * 512, (fh + 1) * 512)
                            for ks in range(DT_TILES):
                                nc.tensor.matmul(psx[:, fs],xpw_t[:, ks, :],xs[ks][:, fs],
                                                 start=(ks == 0), stop=(ks == DT_TILES - 1))
                        nc.scalar.copy(draw, psx[0:16, :])
                        nc.scalar.copy(Brows, psx[32:48, :])
                        nc.scalar.copy(Crows, psx[64:80, :])

                    # ---- dt_proj + softplus -> delta; w = delta * xs ----
                    # softplus = ln(exp(.)+1); all exps batched before all lns
                    # so the act-table is swapped twice, not per-op.
                    delta = []
                    w_t = []
                    es = []
                    with tc.tile_pool(name="pdt", bufs=3, space="PSUM") as pdt, \
                         tc.tile_pool(name="dtp", bufs=1) as dtp:
                        for dt in range(DT_TILES):
                            for fh in range(2):
                                fs = slice(fh * 512, (fh + 1) * 512)
                                ps = pdt.tile([128, 512], F32, tag="pdt", name="pdt")
                                nc.tensor.matmul(ps[:],dtw_t[:, dt * 128:(dt + 1) * 128],draw[:, fs], start=True, stop=True)
                                e = dtp.tile([128, 512], F32, tag=f"de{dt}{fh}", name="de")
                                nc.scalar.activation(e[:], ps[:], AF.Exp,
                                                     bias=dtb_t[:, dt, 0:1])
                                es.append(e)
                        for dt in range(DT_TILES):
                            dl = pers.tile([128, TOK], dt_of("DELTA"), tag=f"delta{dt}", name=f"delta{dt}")
                            for fh in range(2):
                                fs = slice(fh * 512, (fh + 1) * 512)
                                nc.scalar.activation(dl[:, fs], es[dt * 2 + fh][:], AF.Ln, bias=1.0)
                            delta.append(dl)
                            wt = pers.tile([128, 2 * TOK], dt_of("W"), tag=f"w{dt}", name=f"w{dt}")
                            nc.vector.tensor_mul(wt[:, 0:TOK], dl[:], xs[dt][:])
                            nc.vector.tensor_copy(wt[:, TOK:2 * TOK], wt[:, 0:TOK])
                            w_t.append(wt)
                            # poison delta at the scan segment boundaries: the dA
                            # exps then produce exact zeros there (exp(-1000*(n+1))
                            # underflows), so no per-scan memset is needed.  w was
                            # computed above from the clean delta (WAR-ordered).
                            for c in ((0, 512) if not reverse else (511, 1023)):
                                nc.vector.memset(dl[:, c:c + 1], 1000.0)

                    if cfg["PROBE"] == "stop_dt":
                        return [xs[0], xs[1]]
                    # ---- selective scan ----
                    # States are processed in pairs: one [128, 2*TOK] scan row
                    # holds state 2k over tokens [0:TOK] and state 2k+1 over
                    # [TOK:2*TOK]; the decay column is zeroed at the three
                    # internal segment boundaries so a single tensor_tensor_scan
                    # covers 2 states x 2 sequences.
                    idn = wpool.tile([128, 128], BF16, tag="idn", name="idn")
                    from concourse.masks import make_identity
                    make_identity(nc, idn[:])
                    # bounce B/C rows through DRAM; broadcast via stride-0 DMA reads
                    dbc_d = dpool.tile([1, 16, 2, TOK], BF16, tag=f"dbc_d_{s}",
                                       name=f"dbc_d_{s}")
                    nc.sync.dma_start(dbc_d[0:1, :, :, :], dbc[:, :, :])
                    TOK2 = 2 * TOK
                    with tc.tile_pool(name="pyac", bufs=1, space="PSUM") as pyac:
                        y_ps = [pyac.tile([128, TOK], F32, tag=f"yps{dt}", name=f"yps{dt}")
                                for dt in range(DT_TILES)]
                        for k in range(N // 2):
                            B2 = rep.tile([128, TOK2], dt_of("REP"), tag="B2", name="B2", bufs=2)
                            nc.sync.dma_start(
                                B2[:], dbc_d[0:1, 2 * k:2 * k + 2, 0, :].partition_broadcast(128))
                            C2 = rep.tile([128, TOK2], dt_of("REP"), tag="C2", name="C2", bufs=2)
                            nc.sync.dma_start(
                                C2[:], dbc_d[0:1, 2 * k:2 * k + 2, 1, :].partition_broadcast(128))

                            BNDS = (512, 1024, 1536) if not reverse else (511, 1023, 1535)
                            dA2s = []
                            for dt in range(DT_TILES):
                                dA2 = scanw.tile([128, TOK2], dt_of("DA"), tag="dA", name="dA", bufs=4)
                                for j in range(2):
                                    nj = 2 * k + j
                                    nc.scalar.activation(dA2[:, j * TOK:(j + 1) * TOK],
                                                         delta[dt][:], AF.Exp,
                                                         scale=A_t[:, dt, nj:nj + 1])
                                dA2s.append(dA2)
                            for dt in range(DT_TILES):
                                dA2 = dA2s[dt]
                                bx2 = scanw.tile([128, TOK2], dt_of("W"), tag="bx", name="bx", bufs=2)
                                nc.vector.tensor_mul(bx2[:], w_t[dt][:], B2[:])
                                h2 = scanw.tile([128, TOK2], dt_of("H"), tag="h", name="h", bufs=2)
                                if variant == "noscan":
                                    nc.vector.tensor_mul(h2[:], dA2[:], bx2[:])
                                elif not reverse:
                                    nc.vector.tensor_tensor_scan(
                                        h2[:], dA2[:], bx2[:], 0.0, ALU.mult, ALU.add)
                                else:
                                    nc.vector.tensor_tensor_scan(
                                        h2[:], dA2[:, ::-1], bx2[:, ::-1], 0.0,
                                        ALU.mult, ALU.add)
                                p2 = scanw.tile([128, TOK2], dt_of("P"), tag="p", name="p", bufs=2)
                                if not reverse:
                                    nc.vector.tensor_mul(p2[:], h2[:], C2[:])
                                else:
                                    nc.vector.tensor_mul(p2[:], h2[:, ::-1], C2[:])
                                for j in range(2):
                                    for fh in range(2):
                                        sl = slice(j * TOK + fh * 512, j * TOK + (fh + 1) * 512)
                                        fs = slice(fh * 512, (fh + 1) * 512)
                                        nc.tensor.matmul(y_ps[dt][:, fs], idn[:], p2[:, sl],
                                                         start=(k == 0 and j == 0),
                                                         stop=(k == N // 2 - 1 and j == 1))

                        # ---- gate + out_proj + residual ----
                        if cfg["PROBE"] == "stop_scan":
                            return [xs[0], xs[1]]
                        # y = y_ps + Dp*xs, then gate by silu(z) — both in place on xs
                        g = xs
                        for dt in range(DT_TILES):
                            nc.vector.scalar_tensor_tensor(
                                xs[dt][:], xs[dt][:], Dp_t[:, dt, 0:1], y_ps[dt][:],
                                ALU.mult, ALU.add)
                            g_eng = nc.gpsimd if cfg["GP_GATE"] else nc.vector
                            g_eng.tensor_mul(xs[dt][:], xs[dt][:], sz[dt][:])
                    xout = []
                    with tc.tile_pool(name="po", bufs=3, space="PSUM") as po:
                        for m in range(MT):
                            t = pers.tile([128, TOK], BF16, tag=f"x{s}out{m}", name=f"x{s}out{m}")
                            for fh in range(2):
                                fs = slice(fh * 512, (fh + 1) * 512)
                                ps = po.tile([128, 512], F32, tag="po", name="po")
                                for ks in range(DT_TILES):
                                    nc.tensor.matmul(
                                        ps[:],outw_t[:, ks, m * 128:(m + 1) * 128],g[ks][:, fs], start=(ks == 0),
                                        stop=(ks == DT_TILES - 1))
                                nc.vector.tensor_add(t[:, fs], ps[:], xT[m][:, fs])
                            xout.append(t)
                    return xout

                if cfg["PROBE"] == "base":
                    x1 = None
                else:
                    x1 = mamba_layer("f", reverse=False)
                x2 = x1 if (cfg["PROBE"] in ("layer1", "base") or cfg["PROBE"].startswith("stop_")) else mamba_layer("b", reverse=True)

                if cfg["PROBE"] == "base":
                    for m in range(MT):
                        nc.gpsimd.dma_start(outT_d[m * 128:(m + 1) * 128, :], xn[m][:])
                    return
                if cfg["PROBE"] == "nohead" or cfg["PROBE"].startswith("stop_"):
                    for m in range(MT):
                        nc.gpsimd.dma_start(outT_d[m * 128:(m + 1) * 128, :], x1[m][:])
                    return
                # ---- head: relu(cat(x1,x2) @ proj_w + proj_b), residual, layernorm ----
                cat = x1 + x2
                xn2 = []
                with tc.tile_pool(name="ph", bufs=3, space="PSUM") as ph:
                    for m in range(MT):
                        x2n = pers.tile([128, TOK], F32, tag=f"xn2_{m}", name=f"xn2_{m}")
                        for fh in range(2):
                            fs = slice(fh * 512, (fh + 1) * 512)
                            ps = ph.tile([128, 512], F32, tag="ph", name="ph")
                            for ks in range(4):
                                nc.tensor.matmul(
                                    ps[:],projw_t[:, ks, m * 128:(m + 1) * 128],cat[ks][:, fs], start=(ks == 0), stop=(ks == 3))
                            t = work.tile([128, 512], F32, tag="yh", name="yh")
                            nc.scalar.activation(t[:], ps[:], AF.Relu,
                                                 bias=projb_t[:, m, 0:1])
                            nc.vector.tensor_add(x2n[:, fs], t[:], xT[m][:, fs])
                        xn2.append(x2n)

                with tc.tile_pool(name="pln", bufs=1, space="PSUM") as pln:
                    ones_col = wpool.tile([128, 1], F32, tag="ones_col2", name="ones_col2")
                    nc.vector.memset(ones_col[:], 1.0)
                    ones1 = wpool.tile([1, 128], F32, tag="ones1b", name="ones1b")
                    nc.vector.memset(ones1[:], 1.0)
                    mu_ps = pln.tile([1, TOK], F32, tag="mu", name="mu")
                    ss_ps = pln.tile([1, TOK], F32, tag="ss2", name="ss2")
                    for fh in range(2):
                        fs = slice(fh * 512, (fh + 1) * 512)
                        for m in range(MT):
                            nc.tensor.matmul(mu_ps[:, fs],ones_col[:],xn2[m][:, fs],
                                             start=(m == 0), stop=(m == MT - 1))
                            sq = work.tile([128, 512], F32, tag="sqtmp", name="ln_sq")
                            nc.scalar.square(sq[:], xn2[m][:, fs])
                            nc.tensor.matmul(ss_ps[:, fs],ones_col[:],sq[:],
                                             start=(m == 0), stop=(m == MT - 1))
                    mu_row = wpool.tile([1, TOK], F32, tag="mu_row", name="mu_row")
                    nc.scalar.mul(mu_row[:], mu_ps[:], 1.0 / DM)
                    # var = ss/DM - mu^2 (built in rstd_row, then rstd in place)
                    rstd_row = wpool.tile([1, TOK], F32, tag="rstd_row", name="rstd_row")
                    nc.scalar.mul(rstd_row[:], ss_ps[:], 1.0 / DM)
                    mu2 = work.tile([1, TOK], F32, tag="rowtmp", name="mu2")
                    nc.vector.tensor_mul(mu2[:], mu_row[:], mu_row[:])
                    nc.vector.tensor_sub(rstd_row[:], rstd_row[:], mu2[:])
                    eps2 = wpool.tile([1, 1], F32, tag="eps2", name="eps2")
                    nc.vector.memset(eps2[:], 1e-5)
                    nc.scalar.activation(rstd_row[:], rstd_row[:], AF.Ln, bias=eps2[:, 0:1])
                    nc.scalar.activation(rstd_row[:], rstd_row[:], AF.Exp, scale=-0.5)
                    # broadcast mu/rstd rows via DRAM-bounce DMA
                    murs_d = dpool.tile([2, TOK], F32, tag="murs_d", name="murs_d")
                    nc.sync.dma_start(murs_d[0:1, :], mu_row[:])
                    nc.sync.dma_start(murs_d[1:2, :], rstd_row[:])
                    mu_bc = pers.tile([128, TOK], F32, tag="mu_bc", name="mu_bc")
                    nc.sync.dma_start(mu_bc[:], murs_d[0:1, :].partition_broadcast(128))
                    rs2_bc = pers.tile([128, TOK], F32, tag="rs2_bc", name="rs2_bc")
                    nc.sync.dma_start(rs2_bc[:], murs_d[1:2, :].partition_broadcast(128))
                    for m in range(MT):
                        nc.vector.tensor_sub(xn2[m][:], xn2[m][:], mu_bc[:])
                        nc.vector.tensor_mul(xn2[m][:], xn2[m][:], rs2_bc[:])
                        nc.scalar.activation(xn2[m][:], xn2[m][:], AF.Identity,
                                             bias=lnb_t[:, m, 0:1],
                                             scale=lng_t[:, m, 0:1])
                        nc.sync.dma_start(outT_d[m * 128:(m + 1) * 128, :], xn2[m][:])

            if loop_k > 1:
                with tc.For_i(0, loop_k, 1):
                    body()
            else:
                body()

    nc.compile()
    _BUILD_CACHE[key] = nc
    return nc


# ======================================================================
# host entry
# ======================================================================

def _make_in_maps(inputs):
    x = np.asarray(inputs["x"], F32_np)
    fw = _prep_layer_weights(inputs["fm_in"], inputs["fm_convw"], inputs["fm_convb"],
                             inputs["fm_xproj"], inputs["fm_dtw"], inputs["fm_dtb"],
                             inputs["fm_Alog"], inputs["fm_D"], inputs["fm_out"],
                             inputs["fm_norm"])
    bw = _prep_layer_weights(inputs["bm_in"], inputs["bm_convw"], inputs["bm_convb"],
                             inputs["bm_xproj"], inputs["bm_dtw"], inputs["bm_dtb"],
                             inputs["bm_Alog"], inputs["bm_D"], inputs["bm_out"],
                             inputs["bm_norm"])
    sh = _prep_shared_weights(inputs["proj_w"], inputs["proj_b"],
                              inputs["ln_g"], inputs["ln_b"])
    base = {}
    for s, w in (("f", fw), ("b", bw)):
        for k, v in w.items():
            if k in ("convbn", "convb", "inw", "convd", "xpw", "dtw", "dtb", "A", "Dp",
                     "outw"):
                base[f"{s}_{k}"] = v
    base["projw"] = sh["projw"]
    base["projb"] = sh["projb"]
    base["lng"] = sh["lng"]
    base["lnb"] = sh["lnb"]

    in_maps = []
    for c in range(NCORES):
        xc = x[c * BL:(c + 1) * BL]                       # (BL, L, DM)
        xTc = np.ascontiguousarray(xc.reshape(TOK, DM).T)  # (DM, TOK)
        m = dict(base)
        m["xT"] = xTc
        in_maps.append(m)
    return in_maps


def _unshard(results):
    outs = []
    for c in range(NCORES):
        oT = results[c]["outT"]                            # (DM, TOK)
        outs.append(np.ascontiguousarray(oT.T.reshape(BL, L, DM)))
    return np.concatenate(outs, axis=0).astype(F32_np)


def kernel(**inputs):
    from concourse import bass_utils
    nc = _build(loop_k=1)
    in_maps = _make_in_maps(inputs)
    res = bass_utils.run_bass_kernel_spmd(nc, in_maps, core_ids=list(range(NCORES)))
    return _unshard(res.results)



# revision 27
# speedup vs baseline: 1.0185x; 1.0185x over previous
"""Bidirectional Mamba block on 8 Trainium2 NeuronCores (Bass/Tile).

Data-parallel over batch: B=16 -> 2 per core; weights replicated; host gathers.
Per-core layout is feature-major ([feature_partitions, tokens]) with tokens =
batch-major concatenation of the 2 local sequences (t = b*512 + l).

Engines:
  PE   - all projections (weights stationary as lhsT), depthwise causal conv as
         4 accumulating diag-matmuls over shifted views, y = sum_n h_n*C_n
         accumulated in PSUM via identity matmuls (all 4 d-tiles).
  ACT  - silu via the native Silu table; softplus = ln(exp(.)+1);
         rsqrt = exp(-0.5*ln(.)); dA_n = exp(delta * A[:,n]) with per-partition
         scale; PSUM->SBUF copies.
  DVE  - selective scan via tensor_tensor_scan; bx/p elementwise muls (bf16 2x);
         the backward layer feeds the scan with reversed access patterns.
  DMA  - per-token B/C rows are bounced through a DRAM scratch tile and
         partition-broadcast to 128 partitions by pure DMA reads (no engine
         time), replacing one-hot selector matmuls + ACT copies.
"""

import numpy as np

# ---- problem constants (hardcoded per contract) ----
B, L, DM = 16, 512, 256
DI, N, R, KC = 512, 16, 16, 4
NCORES = 8
BL = B // NCORES          # local batch
TOK = BL * L              # 1024 tokens per core
DT_TILES = DI // 128      # 4
MT = DM // 128            # 2
F32_np = np.float32

# ---- dtype knobs for the scan path ----
import ml_dtypes
BF16_np = ml_dtypes.bfloat16

CFG = dict(
    DA="bf16",     # dA (scan decay operand)
    DELTA="bf16",  # delta resident
    W="bf16",      # w = delta*xs (scan drive factor)
    H="bf16",      # scan output h
    REP="bf16",    # B_rep / C_rep broadcast tiles
    P="bf16",      # products h*C
    SZ="bf16",     # silu(z) gate
    XS="bf16",     # conv-silu output / gate buffer
    PROBE="",      # timing probes: shrink a stage's work (breaks numerics)
    P_POOL_DT=(),       # d-tiles whose p=h*C mul runs on GPSIMD
    BX_POOL_DT=(),      # d-tiles whose bx=w*B mul runs on GPSIMD
    GP_GATE=0,          # gate mul xs*sz on GPSIMD
)

_BUILD_CACHE = {}


# ======================================================================
# host-side weight preparation
# ======================================================================

def _prep_layer_weights(inw, convw, convb, xprojw, dtw, dtb, Alog, Dp, outw, normw):
    """Fold/reshape one mamba layer's weights into device layouts."""
    out = {}
    # in_proj with rmsnorm weight folded into rows: [128, 2, 1024]
    w = (np.asarray(normw)[:, None] * np.asarray(inw)).astype(F32_np)
    out["inw"] = np.ascontiguousarray(w.reshape(2, 128, 2 * DI).transpose(1, 0, 2)).astype(BF16_np)
    # conv diag matrices: [128, 16(dt*4+k), 128]
    cd = np.zeros((128, DT_TILES * KC, 128), F32_np)
    cw = np.asarray(convw).astype(F32_np)  # (KC, 1, DI)
    for dt in range(DT_TILES):
        for k in range(KC):
            idx = np.arange(128)
            cd[idx, dt * KC + k, idx] = cw[k, 0, dt * 128 + idx]
    out["convd"] = np.ascontiguousarray(cd).astype(BF16_np)
    out["convbn"] = np.ascontiguousarray(
        (-np.asarray(convb).astype(F32_np)).reshape(DT_TILES, 128, 1).transpose(1, 0, 2))
    out["convb"] = np.ascontiguousarray(
        np.asarray(convb).astype(F32_np).reshape(DT_TILES, 128, 1).transpose(1, 0, 2))
    # xproj padded so delta_raw/B/C land at partitions 0/32/64: [128, 4, 96]
    xp = np.zeros((DI, 96), F32_np)
    xpw = np.asarray(xprojw).astype(F32_np)
    xp[:, 0:R] = xpw[:, 0:R]
    xp[:, 32:32 + N] = xpw[:, R:R + N]
    xp[:, 64:64 + N] = xpw[:, R + N:R + 2 * N]
    out["xpw"] = np.ascontiguousarray(xp.reshape(DT_TILES, 128, 96).transpose(1, 0, 2)).astype(BF16_np)
    out["dtw"] = np.ascontiguousarray(np.asarray(dtw).astype(F32_np)).astype(BF16_np)          # (16, 512)
    out["dtb"] = np.ascontiguousarray(
        np.asarray(dtb).astype(F32_np).reshape(DT_TILES, 128, 1).transpose(1, 0, 2))
    A = (-np.exp(np.asarray(Alog).astype(np.float64))).astype(F32_np)          # (512, 16)
    out["A"] = np.ascontiguousarray(A.reshape(DT_TILES, 128, N).transpose(1, 0, 2))
    out["Dp"] = np.ascontiguousarray(
        np.asarray(Dp).astype(F32_np).reshape(DT_TILES, 128, 1).transpose(1, 0, 2))
    out["outw"] = np.ascontiguousarray(
        np.asarray(outw).astype(F32_np).reshape(DT_TILES, 128, DM).transpose(1, 0, 2)).astype(BF16_np)
    return out


def _prep_shared_weights(proj_w, proj_b, ln_g, ln_b):
    out = {}
    out["projw"] = np.ascontiguousarray(
        np.asarray(proj_w).astype(F32_np).reshape(4, 128, DM).transpose(1, 0, 2)).astype(BF16_np)
    out["projb"] = np.ascontiguousarray(
        np.asarray(proj_b).astype(F32_np).reshape(MT, 128, 1).transpose(1, 0, 2))
    out["lng"] = np.ascontiguousarray(
        np.asarray(ln_g).astype(F32_np).reshape(MT, 128, 1).transpose(1, 0, 2))
    out["lnb"] = np.ascontiguousarray(
        np.asarray(ln_b).astype(F32_np).reshape(MT, 128, 1).transpose(1, 0, 2))
    return out


# ======================================================================
# device program
# ======================================================================

def _build(loop_k=1, cfg=None, variant="full"):
    cfg = dict(CFG if cfg is None else cfg)
    key = (loop_k, variant, tuple(sorted(cfg.items())))
    if key in _BUILD_CACHE:
        return _BUILD_CACHE[key]

    import concourse.bacc as bacc
    import concourse.mybir as mybir
    import concourse.tile as tile

    F32 = mybir.dt.float32
    BF16 = mybir.dt.bfloat16
    AF = mybir.ActivationFunctionType
    ALU = mybir.AluOpType
    AX = mybir.AxisListType

    def dt_of(kname):
        return F32 if cfg[kname] == "f32" else BF16

    nc = bacc.Bacc("TRN2", target_bir_lowering=False, debug=False)

    def din(name, shape, dt=None):
        return nc.dram_tensor(name, list(shape), dt or F32, kind="ExternalInput").ap()

    # --- DRAM I/O ---
    xT_d = din("xT", (DM, TOK))
    lw_d = {}
    for s in ("f", "b"):
        lw_d[s] = {
            "inw": din(f"{s}_inw", (128, 2, 2 * DI), BF16),
            "convd": din(f"{s}_convd", (128, DT_TILES * KC, 128), BF16),
            "convbn": din(f"{s}_convbn", (128, DT_TILES, 1)),
            "convb": din(f"{s}_convb", (128, DT_TILES, 1)),
            "xpw": din(f"{s}_xpw", (128, DT_TILES, 96), BF16),
            "dtw": din(f"{s}_dtw", (16, DI), BF16),
            "dtb": din(f"{s}_dtb", (128, DT_TILES, 1)),
            "A": din(f"{s}_A", (128, DT_TILES, N)),
            "Dp": din(f"{s}_Dp", (128, DT_TILES, 1)),
            "outw": din(f"{s}_outw", (128, DT_TILES, DM), BF16),
        }
    projw_d = din("projw", (128, 4, DM), BF16)
    projb_d = din("projb", (128, MT, 1))
    lng_d = din("lng", (128, MT, 1))
    lnb_d = din("lnb", (128, MT, 1))
    outT_d = nc.dram_tensor("outT", [DM, TOK], F32, kind="ExternalOutput").ap()

    PAD = KC - 1  # 3
    CONVW = 2 * PAD + L  # padded per-batch row length 518

    with tile.TileContext(nc) as tc:
        from contextlib import ExitStack
        with ExitStack() as ctx:
            wpool = ctx.enter_context(tc.tile_pool(name="wpool", bufs=1))
            pers = ctx.enter_context(tc.tile_pool(name="pers", bufs=1))
            work = ctx.enter_context(tc.tile_pool(name="work", bufs=2))
            rep = ctx.enter_context(tc.tile_pool(name="rep", bufs=3))
            scanw = ctx.enter_context(tc.tile_pool(name="scanw", bufs=3))
            dpool = ctx.enter_context(tc.tile_pool(name="dpool", bufs=1, space="DRAM"))

            def body():
                # ---- load shared weights ----
                projw_t = wpool.tile([128, 4, DM], BF16, tag="projw", name="projw")
                nc.sync.dma_start(projw_t[:], projw_d[:])
                projb_t = wpool.tile([128, MT, 1], F32, tag="projb", name="projb")
                nc.sync.dma_start(projb_t[:], projb_d[:])
                lng_t = wpool.tile([128, MT, 1], F32, tag="lng", name="lng")
                nc.sync.dma_start(lng_t[:], lng_d[:])
                lnb_t = wpool.tile([128, MT, 1], F32, tag="lnb", name="lnb")
                nc.sync.dma_start(lnb_t[:], lnb_d[:])

                xT = []
                for m in range(MT):
                    t = pers.tile([128, TOK], F32, tag=f"xT{m}", name=f"xT{m}")
                    nc.sync.dma_start(t[:], xT_d[m * 128:(m + 1) * 128, :])
                    xT.append(t)

                # ---- shared RMSNorm: xn = x * rsqrt(mean(x^2) + eps) ----
                xn = []
                with tc.tile_pool(name="prms", bufs=1, space="PSUM") as prms:
                    ones_col = wpool.tile([128, 1], F32, tag="ones_col", name="ones_col")
                    nc.vector.memset(ones_col[:], 1.0)
                    ss_ps = prms.tile([1, TOK], F32, tag="ss", name="ss")
                    for fh in range(2):
                        fs = slice(fh * 512, (fh + 1) * 512)
                        for m in range(MT):
                            sq = work.tile([128, 512], F32, tag="sqtmp", name="rms_sq")
                            nc.scalar.square(sq[:], xT[m][:, fs])
                            nc.tensor.matmul(ss_ps[:, fs],ones_col[:],sq[:],
                                             start=(m == 0), stop=(m == MT - 1))
                    # rs = exp(-0.5 * ln(ss/DM + eps))
                    eps1 = wpool.tile([1, 1], F32, tag="eps1", name="eps1")
                    nc.vector.memset(eps1[:], 1e-5)
                    rs_row = work.tile([1, TOK], F32, tag="rowtmp", name="rs_row")
                    nc.scalar.activation(rs_row[:], ss_ps[:], AF.Ln,
                                         scale=1.0 / DM, bias=eps1[:, 0:1])
                    nc.scalar.activation(rs_row[:], rs_row[:], AF.Exp, scale=-0.5)
                    # broadcast rs to 128 partitions via DRAM-bounce DMA
                    rs_d = dpool.tile([1, TOK], F32, tag="rs_d", name="rs_d")
                    nc.sync.dma_start(rs_d[:], rs_row[:])
                    rs_bc = pers.tile([128, TOK], F32, tag="rs_bc", name="rs_bc")
                    nc.sync.dma_start(rs_bc[:], rs_d[0:1, :].partition_broadcast(128))
                    for m in range(MT):
                        t = pers.tile([128, TOK], BF16, tag=f"xn{m}", name=f"xn{m}")
                        nc.gpsimd.tensor_mul(t[:], xT[m][:], rs_bc[:])
                        xn.append(t)

                # ---- one mamba layer ----
                def mamba_layer(s, reverse):
                    W = lw_d[s]
                    inw_t = wpool.tile([128, 2, 2 * DI], BF16, tag="inw", name="inw")
                    nc.sync.dma_start(inw_t[:], W["inw"][:])
                    convd_t = wpool.tile([128, DT_TILES * KC, 128], BF16, tag="convd", name="convd")
                    nc.sync.dma_start(convd_t[:], W["convd"][:])
                    convb_t = wpool.tile([128, DT_TILES, 1], F32, tag="convb", name="convb")
                    nc.sync.dma_start(convb_t[:], W["convb"][:])
                    xpw_t = wpool.tile([128, DT_TILES, 96], BF16, tag="xpw", name="xpw")
                    nc.sync.dma_start(xpw_t[:], W["xpw"][:])
                    dtw_t = wpool.tile([16, DI], BF16, tag="dtw", name="dtw")
                    nc.sync.dma_start(dtw_t[:], W["dtw"][:])
                    dtb_t = wpool.tile([128, DT_TILES, 1], F32, tag="dtb", name="dtb")
                    nc.sync.dma_start(dtb_t[:], W["dtb"][:])
                    A_t = wpool.tile([128, DT_TILES, N], F32, tag="A", name="A")
                    nc.sync.dma_start(A_t[:], W["A"][:])
                    Dp_t = wpool.tile([128, DT_TILES, 1], F32, tag="Dp", name="Dp")
                    nc.sync.dma_start(Dp_t[:], W["Dp"][:])
                    outw_t = wpool.tile([128, DT_TILES, DM], BF16, tag="outw", name="outw")
                    nc.sync.dma_start(outw_t[:], W["outw"][:])

                    xmpad = []
                    sz = []
                    xs = []
                    for dt in range(DT_TILES):
                        t = pers.tile([128, BL, CONVW], BF16, tag=f"xmpad{dt}", name=f"xmpad{dt}")
                        nc.vector.memset(t[:, :, 0:PAD], 0.0)
                        nc.vector.memset(t[:, :, PAD + L:CONVW], 0.0)
                        xmpad.append(t)
                        sz.append(pers.tile([128, TOK], dt_of("SZ"), tag=f"sz{dt}", name=f"sz{dt}"))
                        xs.append(pers.tile([128, TOK], dt_of("XS"), tag=f"xs{dt}", name=f"xs{dt}"))

                    # ---- in_proj ----
                    with tc.tile_pool(name="pp", bufs=4, space="PSUM") as pp:
                        for m in range(8):
                            for fh in range(2):
                                fs = slice(fh * 512, (fh + 1) * 512)
                                ps = pp.tile([128, 512], F32, tag="pp", name="pp")
                                for ks in range(2):
                                    nc.tensor.matmul(
                                        ps[:],inw_t[:, ks, m * 128:(m + 1) * 128],xn[ks][:, fs], start=(ks == 0), stop=(ks == 1))
                                if m < 4:
                                    # xm -> padded conv buffer (fh == local batch idx)
                                    nc.scalar.copy(xmpad[m][:, fh, PAD:PAD + L], ps[:])
                                else:
                                    zdt = m - 4
                                    nc.scalar.activation(sz[zdt][:, fs], ps[:], AF.Silu)

                        # ---- depthwise causal conv + silu ----
                        for dt in range(DT_TILES):
                            for b in range(BL):
                                ps = pp.tile([128, 512], F32, tag="pp", name="pp")
                                for k in range(KC):
                                    off = k if not reverse else (2 * PAD - k)
                                    nc.tensor.matmul(
                                        ps[:],convd_t[:, dt * KC + k, :],xmpad[dt][:, b, off:off + L],
                                        start=(k == 0), stop=(k == KC - 1))
                                bs = slice(b * L, (b + 1) * L)
                                nc.scalar.activation(xs[dt][:, bs], ps[:], AF.Silu,
                                                     bias=convb_t[:, dt, 0:1])

                    if cfg["PROBE"] == "stop_conv":
                        return [xs[0], xs[1]]
                    # ---- xproj -> delta_raw / Brows / Crows ----
                    dbc = pers.tile([16, 2, TOK], BF16, tag="dbc", name="dbc")
                    draw_t = work.tile([16, TOK], BF16, tag="draw", name="draw_t")
                    draw = draw_t[:, :]
                    Brows = dbc[:, 0, :]
                    Crows = dbc[:, 1, :]
                    with tc.tile_pool(name="pxp", bufs=1, space="PSUM") as pxp:
                        psx = pxp.tile([96, TOK], F32, tag="pxp", name="pxp")
                        for fh in range(2):
                            fs = slice(fh * 512, (fh + 1) * 512)
                            for ks in range(DT_TILES):
                                nc.tensor.matmul(psx[:, fs],xpw_t[:, ks, :],xs[ks][:, fs],
                                                 start=(ks == 0), stop=(ks == DT_TILES - 1))
                        nc.scalar.copy(draw, psx[0:16, :])
                        nc.scalar.copy(Brows, psx[32:48, :])
                        nc.scalar.copy(Crows, psx[64:80, :])

                    # ---- dt_proj + softplus -> delta; w = delta * xs ----
                    # softplus = ln(exp(.)+1); all exps batched before all lns
                    # so the act-table is swapped twice, not per-op.
                    delta = []
                    w_t = []
                    es = []
                    with tc.tile_pool(name="pdt", bufs=3, space="PSUM") as pdt, \
                         tc.tile_pool(name="dtp", bufs=1) as dtp:
                        for dt in range(DT_TILES):
                            for fh in range(2):
                                fs = slice(fh * 512, (fh + 1) * 512)
                                ps = pdt.tile([128, 512], F32, tag="pdt", name="pdt")
                                nc.tensor.matmul(ps[:],dtw_t[:, dt * 128:(dt + 1) * 128],draw[:, fs], start=True, stop=True)
                                e = dtp.tile([128, 512], F32, tag=f"de{dt}{fh}", name="de")
                                nc.scalar.activation(e[:], ps[:], AF.Exp,
                                                     bias=dtb_t[:, dt, 0:1])
                                es.append(e)
                        for dt in range(DT_TILES):
                            dl = pers.tile([128, TOK], dt_of("DELTA"), tag=f"delta{dt}", name=f"delta{dt}")
                            for fh in range(2):
                                fs = slice(fh * 512, (fh + 1) * 512)
                                nc.scalar.activation(dl[:, fs], es[dt * 2 + fh][:], AF.Ln, bias=1.0)
                            delta.append(dl)
                            wt = pers.tile([128, TOK], dt_of("W"), tag=f"w{dt}", name=f"w{dt}")
                            nc.vector.tensor_mul(wt[:], dl[:], xs[dt][:])
                            w_t.append(wt)

                    if cfg["PROBE"] == "stop_dt":
                        return [xs[0], xs[1]]
                    # ---- selective scan ----
                    idn = wpool.tile([128, 128], BF16, tag="idn", name="idn")
                    from concourse.masks import make_identity
                    make_identity(nc, idn[:])
                    # bounce B/C rows through DRAM; broadcast via stride-0 DMA reads
                    dbc_d = dpool.tile([16, 2, TOK], BF16, tag=f"dbc_d_{s}",
                                       name=f"dbc_d_{s}")
                    nc.sync.dma_start(dbc_d[:], dbc[:])
                    with tc.tile_pool(name="pyac", bufs=1, space="PSUM") as pyac:
                        y_ps = [pyac.tile([128, TOK], F32, tag=f"yps{dt}", name=f"yps{dt}")
                                for dt in range(DT_TILES)]
                        for n in range(N):
                            BC = rep.tile([128, 2, TOK], dt_of("REP"), tag="BC", name="BC")
                            nc.sync.dma_start(
                                BC[:], dbc_d[n:n + 1, :, :].partition_broadcast(128))
                            B_rep = BC[:, 0, :]
                            C_rep = BC[:, 1, :]

                            # decay column zeroed at the local-batch boundary so a
                            # single scan spans both sequences (fresh state at the
                            # second segment's first step)
                            BND = L if not reverse else L - 1
                            dAs = []
                            for dt in range(DT_TILES):
                                dA = scanw.tile([128, TOK], dt_of("DA"), tag="dA", name="dA", bufs=6)
                                _dsl = slice(0, 64) if cfg["PROBE"] in ("dA", "acts") else slice(0, TOK)
                                nc.scalar.activation(dA[:, _dsl], delta[dt][:, _dsl], AF.Exp,
                                                     scale=A_t[:, dt, n:n + 1])
                                nc.gpsimd.memset(dA[:, BND:BND + 1], 0.0)
                                dAs.append(dA)
                            for dt in range(DT_TILES):
                                dA = dAs[dt]
                                bx = scanw.tile([128, TOK], dt_of("W"), tag="bx", name="bx")
                                _bsl = slice(0, 64) if cfg["PROBE"] == "tt" else slice(0, TOK)
                                bx_eng = nc.gpsimd if dt in cfg["BX_POOL_DT"] else nc.vector
                                bx_eng.tensor_mul(bx[:, _bsl], w_t[dt][:, _bsl], B_rep[:, _bsl])
                                h = scanw.tile([128, TOK], dt_of("H"), tag="h", name="h")
                                if variant == "noscan":
                                    nc.vector.tensor_mul(h[:], dA[:], bx[:])
                                elif not reverse:
                                    nc.vector.tensor_tensor_scan(
                                        h[:], dA[:], bx[:], 0.0, ALU.mult, ALU.add)
                                else:
                                    nc.vector.tensor_tensor_scan(
                                        h[:], dA[:, ::-1], bx[:, ::-1], 0.0,
                                        ALU.mult, ALU.add)
                                p = scanw.tile([128, TOK], dt_of("P"), tag="p", name="p")
                                p_eng = nc.gpsimd if dt in cfg["P_POOL_DT"] else nc.vector
                                if not reverse:
                                    p_eng.tensor_mul(p[:], h[:], C_rep[:])
                                else:
                                    p_eng.tensor_mul(p[:], h[:, ::-1], C_rep[:])
                                for fh in range(2):
                                    fs = slice(fh * 512, (fh + 1) * 512)
                                    nc.tensor.matmul(y_ps[dt][:, fs], idn[:], p[:, fs],
                                                     start=(n == 0), stop=(n == N - 1))

                        # ---- gate + out_proj + residual ----
                        if cfg["PROBE"] == "stop_scan":
                            return [xs[0], xs[1]]
                        # y = y_ps + Dp*xs, then gate by silu(z) — both in place on xs
                        g = xs
                        for dt in range(DT_TILES):
                            nc.vector.scalar_tensor_tensor(
                                xs[dt][:], xs[dt][:], Dp_t[:, dt, 0:1], y_ps[dt][:],
                                ALU.mult, ALU.add)
                            g_eng = nc.gpsimd if cfg["GP_GATE"] else nc.vector
                            g_eng.tensor_mul(xs[dt][:], xs[dt][:], sz[dt][:])
                    xout = []
                    with tc.tile_pool(name="po", bufs=3, space="PSUM") as po:
                        for m in range(MT):
                            t = pers.tile([128, TOK], BF16, tag=f"x{s}out{m}", name=f"x{s}out{m}")
                            for fh in range(2):
                                fs = slice(fh * 512, (fh + 1) * 512)
                                ps = po.tile([128, 512], F32, tag="po", name="po")
                                for ks in range(DT_TILES):
                                    nc.tensor.matmul(
                                        ps[:],outw_t[:, ks, m * 128:(m + 1) * 128],g[ks][:, fs], start=(ks == 0),
                                        stop=(ks == DT_TILES - 1))
                                nc.vector.tensor_add(t[:, fs], ps[:], xT[m][:, fs])
                            xout.append(t)
                    return xout

                if cfg["PROBE"] == "base":
                    x1 = None
                else:
                    x1 = mamba_layer("f", reverse=False)
                x2 = x1 if (cfg["PROBE"] in ("layer1", "base") or cfg["PROBE"].startswith("stop_")) else mamba_layer("b", reverse=True)

                if cfg["PROBE"] == "base":
                    for m in range(MT):
                        nc.gpsimd.dma_start(outT_d[m * 128:(m + 1) * 128, :], xn[m][:])
                    return
                if cfg["PROBE"] == "nohead" or cfg["PROBE"].startswith("stop_"):
                    for m in range(MT):
                        nc.gpsimd.dma_start(outT_d[m * 128:(m + 1) * 128, :], x1[m][:])
                    return
                # ---- head: relu(cat(x1,x2) @ proj_w + proj_b), residual, layernorm ----
                cat = x1 + x2
                xn2 = []
                with tc.tile_pool(name="ph", bufs=3, space="PSUM") as ph:
                    for m in range(MT):
                        x2n = pers.tile([128, TOK], F32, tag=f"xn2_{m}", name=f"xn2_{m}")
                        for fh in range(2):
                            fs = slice(fh * 512, (fh + 1) * 512)
                            ps = ph.tile([128, 512], F32, tag="ph", name="ph")
                            for ks in range(4):
                                nc.tensor.matmul(
                                    ps[:],projw_t[:, ks, m * 128:(m + 1) * 128],cat[ks][:, fs], start=(ks == 0), stop=(ks == 3))
                            t = work.tile([128, 512], F32, tag="yh", name="yh")
                            nc.scalar.activation(t[:], ps[:], AF.Relu,
                                                 bias=projb_t[:, m, 0:1])
                            nc.vector.tensor_add(x2n[:, fs], t[:], xT[m][:, fs])
                        xn2.append(x2n)

                with tc.tile_pool(name="pln", bufs=1, space="PSUM") as pln:
                    ones_col = wpool.tile([128, 1], F32, tag="ones_col2", name="ones_col2")
                    nc.vector.memset(ones_col[:], 1.0)
                    ones1 = wpool.tile([1, 128], F32, tag="ones1b", name="ones1b")
                    nc.vector.memset(ones1[:], 1.0)
                    mu_ps = pln.tile([1, TOK], F32, tag="mu", name="mu")
                    ss_ps = pln.tile([1, TOK], F32, tag="ss2", name="ss2")
                    for fh in range(2):
                        fs = slice(fh * 512, (fh + 1) * 512)
                        for m in range(MT):
                            nc.tensor.matmul(mu_ps[:, fs],ones_col[:],xn2[m][:, fs],
                                             start=(m == 0), stop=(m == MT - 1))
                            sq = work.tile([128, 512], F32, tag="sqtmp", name="ln_sq")
                            nc.scalar.square(sq[:], xn2[m][:, fs])
                            nc.tensor.matmul(ss_ps[:, fs],ones_col[:],sq[:],
                                             start=(m == 0), stop=(m == MT - 1))
                    mu_row = wpool.tile([1, TOK], F32, tag="mu_row", name="mu_row")
                    nc.scalar.mul(mu_row[:], mu_ps[:], 1.0 / DM)
                    # var = ss/DM - mu^2 (built in rstd_row, then rstd in place)
                    rstd_row = wpool.tile([1, TOK], F32, tag="rstd_row", name="rstd_row")
                    nc.scalar.mul(rstd_row[:], ss_ps[:], 1.0 / DM)
                    mu2 = work.tile([1, TOK], F32, tag="rowtmp", name="mu2")
                    nc.vector.tensor_mul(mu2[:], mu_row[:], mu_row[:])
                    nc.vector.tensor_sub(rstd_row[:], rstd_row[:], mu2[:])
                    eps2 = wpool.tile([1, 1], F32, tag="eps2", name="eps2")
                    nc.vector.memset(eps2[:], 1e-5)
                    nc.scalar.activation(rstd_row[:], rstd_row[:], AF.Ln, bias=eps2[:, 0:1])
                    nc.scalar.activation(rstd_row[:], rstd_row[:], AF.Exp, scale=-0.5)
                    # broadcast mu/rstd rows via DRAM-bounce DMA
                    murs_d = dpool.tile([2, TOK], F32, tag="murs_d", name="murs_d")
                    nc.sync.dma_start(murs_d[0:1, :], mu_row[:])
                    nc.sync.dma_start(murs_d[1:2, :], rstd_row[:])
                    mu_bc = pers.tile([128, TOK], F32, tag="mu_bc", name="mu_bc")
                    nc.sync.dma_start(mu_bc[:], murs_d[0:1, :].partition_broadcast(128))
                    rs2_bc = pers.tile([128, TOK], F32, tag="rs2_bc", name="rs2_bc")
                    nc.sync.dma_start(rs2_bc[:], murs_d[1:2, :].partition_broadcast(128))
                    for m in range(MT):
                        nc.gpsimd.tensor_sub(xn2[m][:], xn2[m][:], mu_bc[:])
                        nc.gpsimd.tensor_mul(xn2[m][:], xn2[m][:], rs2_bc[:])
                        nc.scalar.activation(xn2[m][:], xn2[m][:], AF.Identity,
                                             bias=lnb_t[:, m, 0:1],
                                             scale=lng_t[:, m, 0:1])
                        nc.sync.dma_start(outT_d[m * 128:(m + 1) * 128, :], xn2[m][:])

            if loop_k > 1:
                with tc.For_i(0, loop_k, 1):
                    body()
            else:
                body()

    nc.compile()
    _BUILD_CACHE[key] = nc
    return nc


# ======================================================================
# host entry
# ======================================================================

def _make_in_maps(inputs):
    x = np.asarray(inputs["x"], F32_np)
    fw = _prep_layer_weights(inputs["fm_in"], inputs["fm_convw"], inputs["fm_convb"],
                             inputs["fm_xproj"], inputs["fm_dtw"], inputs["fm_dtb"],
                             inputs["fm_Alog"], inputs["fm_D"], inputs["fm_out"],
                             inputs["fm_norm"])
    bw = _prep_layer_weights(inputs["bm_in"], inputs["bm_convw"], inputs["bm_convb"],
                             inputs["bm_xproj"], inputs["bm_dtw"], inputs["bm_dtb"],
                             inputs["bm_Alog"], inputs["bm_D"], inputs["bm_out"],
                             inputs["bm_norm"])
    sh = _prep_shared_weights(inputs["proj_w"], inputs["proj_b"],
                              inputs["ln_g"], inputs["ln_b"])
    base = {}
    for s, w in (("f", fw), ("b", bw)):
        for k, v in w.items():
            if k in ("convbn", "convb", "inw", "convd", "xpw", "dtw", "dtb", "A", "Dp",
                     "outw"):
                base[f"{s}_{k}"] = v
    base["projw"] = sh["projw"]
    base["projb"] = sh["projb"]
    base["lng"] = sh["lng"]
    base["lnb"] = sh["lnb"]

    in_maps = []
    for c in range(NCORES):
        xc = x[c * BL:(c + 1) * BL]                       # (BL, L, DM)
        xTc = np.ascontiguousarray(xc.reshape(TOK, DM).T)  # (DM, TOK)
        m = dict(base)
        m["xT"] = xTc
        in_maps.append(m)
    return in_maps


def _unshard(results):
    outs = []
    for c in range(NCORES):
        oT = results[c]["outT"]                            # (DM, TOK)
        outs.append(np.ascontiguousarray(oT.T.reshape(BL, L, DM)))
    return np.concatenate(outs, axis=0).astype(F32_np)


def kernel(**inputs):
    from concourse import bass_utils
    nc = _build(loop_k=1)
    in_maps = _make_in_maps(inputs)
    res = bass_utils.run_bass_kernel_spmd(nc, in_maps, core_ids=list(range(NCORES)))
    return _unshard(res.results)

